# revision 1
# baseline (speedup 1.0000x reference)
"""Trainium2 Bass kernel for deterministic NeuralSort soft-kNN (DKNN).

Math (per query b over N neighbors):
    s_j   = -||q_b - x_j||^2
    A_j   = sum_i |s_j - s_i|
    P[r,j]= softmax_j(scaling[r] * s_j - A_j),  r = 0..K-1, scaling[r] = N+1-2(r+1)
    out_j = sum_r P[r,j]

Key reduction: s_j = u_j - ||q_b||^2 with u_j = 2 q_b.x_j - ||x_j||^2.  The
||q||^2 term is constant in j, so it cancels in A (pairwise diffs) and shifts
every softmax row by a constant (scaling[r]*||q||^2) which softmax ignores.
So we never compute ||q||^2.

Sharding: data-parallel over the B=128 queries across 8 cores (16 each);
neighbors replicated.

Per-core hot loop (the O(B_local * N^2) part): for each query b, broadcast
u_b to 128 partitions (DMA), then for each 128-row block of pairwise rows,
one fused op produces |u_j - u_p| with the row-sum accumulated on the fly:
  - ScalarE:  activation(Abs, bias=-u_p, accum_out)      (3 blocks / query)
  - VectorE:  tensor_scalar(add -u_p, abs_max 0, accum)  (5 blocks / query)
By symmetry of |u_j - u_i| the free-dim row sums ARE A_sum for the block's
partition indices, so no cross-partition reduce is needed.
"""

import numpy as np

import concourse.bass as bass
import concourse.bacc as bacc
import concourse.tile as tile
from concourse import mybir
from concourse.masks import make_identity
from concourse.bass_utils import run_bass_kernel_spmd

AFT = mybir.ActivationFunctionType
ALU = mybir.AluOpType
FP32 = mybir.dt.float32
BF16 = mybir.dt.bfloat16

B, N, D, TOPK = 128, 1024, 128, 10
NCORES = 8
BL = B // NCORES          # 16 queries per core
NBLK = N // 128           # 8 row-blocks of the pairwise matrix
GROUPS = 2                # softmax groups (8 queries x 10 rows = 80 partitions)
GB = BL // GROUPS         # 8

# Static engine split of the queries (ratio ~ ACT vs DVE+PE per-query cost).
# ACT queries: fused Abs+accum on ScalarE (self-contained, A in transposed form).
# DVE queries: tensor_scalar |diff| tiles on VectorE, row-reduced by TensorE
# selector-matmuls straight into a row-form PSUM accumulator.
# ACT queries spread across both softmax groups so ScalarE stays busy through
# the whole pairwise phase; groups are {0..7} and {8..15}, group 0's queries
# are scheduled first so its softmax overlaps group 1's pairwise work.
ACT_SET = (0, 1, 2, 8, 9)
G0_DVE = (3, 4, 5, 6, 7)
G1_DVE = (10, 11, 12, 13, 14, 15)
GPS_SET = (10, 12)   # whole queries whose G-pass runs on GpSimd
TAIL_SPLIT = (13, 14, 15)  # final-pair queries: blocks t>=5 go to GpSimd too
PAIRS = ((0, 3), (1, 4), (2, 5), (6, 7), (8, 10), (9, 11), (12, 13), (14, 15))


def _host_consts():
    scaling = (N + 1 - 2.0 * (np.arange(TOPK) + 1)).astype(np.float32)
    E = np.zeros((BL, GROUPS, GB, TOPK), np.float32)
    F = np.zeros((BL, GROUPS, GB, TOPK), np.float32)
    for g in range(GROUPS):
        for bl in range(GB):
            E[g * GB + bl, g, bl, :] = scaling
            F[g * GB + bl, g, bl, :] = -1.0
    G = np.zeros((GB * TOPK, GB), np.float32)
    for bl in range(GB):
        G[bl * TOPK : (bl + 1) * TOPK, bl] = 1.0
    F = F.reshape(BL, -1)
    Fa = F.copy()
    Fd = F.copy()
    for b in range(BL):
        (Fd if b in ACT_SET else Fa)[b, :] = 0.0
    return E.reshape(BL, -1), Fa, Fd, G


def _build_nc(debug_taps=False):
    nc = bacc.Bacc(None, target_bir_lowering=False)

    q_in = nc.dram_tensor("query", [BL, D], FP32, kind="ExternalInput")
    x_in = nc.dram_tensor("neighbors", [N, D], FP32, kind="ExternalInput")
    out_t = nc.dram_tensor("out", [BL, N], FP32, kind="ExternalOutput")
    if debug_taps:
        dbg_u = nc.dram_tensor("dbg_u", [BL, N], FP32, kind="ExternalOutput")
        dbg_a = nc.dram_tensor("dbg_a", [BL, N], FP32, kind="ExternalOutput")
        dbg_nut = nc.dram_tensor("dbg_nut", [128, NBLK * BL], FP32, kind="ExternalOutput")
        dbg_paw = nc.dram_tensor("dbg_paw", [80, N], FP32, kind="ExternalOutput")

    E, Fa, Fd, G = _host_consts()
    e_in = nc.inline_tensor(E, "lhs_e")
    fa_in = nc.inline_tensor(Fa, "lhs_fa")
    fd_in = nc.inline_tensor(Fd, "lhs_fd")
    g_in = nc.inline_tensor(G, "lhs_g")

    with tile.TileContext(nc) as tc:
        with (
            tc.tile_pool(name="consts", bufs=1) as consts,
            tc.tile_pool(name="xp", bufs=1) as xp,
            tc.tile_pool(name="bcast", bufs=4) as bcast,
            tc.tile_pool(name="scrA", bufs=2) as scrA,
            tc.tile_pool(name="scrD", bufs=3) as scrD,
            tc.tile_pool(name="scrP", bufs=2) as scrP,
            tc.tile_pool(name="cmbp", bufs=2) as cmbp,
            tc.tile_pool(name="expp", bufs=2) as expp,
            tc.tile_pool(name="small", bufs=8) as small,
            tc.tile_pool(name="dramp", bufs=1, space="DRAM") as dramp,
        ):
            ident = consts.tile([128, 128], FP32)
            make_identity(nc, ident)
            ones128 = consts.tile([128, 1], FP32)
            nc.vector.memset(ones128, 1.0)
            ones1xb = consts.tile([1, BL], FP32)
            nc.vector.memset(ones1xb, 1.0)

            # ---- Phase A: neighbors in, transpose to [d, j]; row norms ----
            x_sb = xp.tile([128, NBLK, D], FP32)
            xv = x_in[:].rearrange("(t p) d -> p t d", p=128)
            half = NBLK // 2
            nc.default_dma_engine.dma_start(out=x_sb[:, :half, :], in_=xv[:, :half, :])
            q_sb = small.tile([BL, D], FP32)
            nc.default_dma_engine.dma_start(out=q_sb, in_=q_in[:])
            nc.default_dma_engine.dma_start(out=x_sb[:, half:, :], in_=xv[:, half:, :])

            xT = xp.tile([128, N], FP32)  # xT[d, j] = X[j, d]
            with tc.tile_pool(name="ps_tr", bufs=2, space="PSUM") as ps_tr:
                for t in range(NBLK):
                    ptr = ps_tr.tile([128, 128], FP32)
                    nc.tensor.transpose(ptr, x_sb[:, t, :], ident)
                    nc.any.tensor_copy(xT[:, t * 128 : (t + 1) * 128], ptr)

            sq = xp.tile([128, N], FP32)
            negx2 = consts.tile([1, N], FP32)  # -||x_j||^2
            with tc.tile_pool(name="ps_x2", bufs=1, space="PSUM") as ps_x2:
                px2 = ps_x2.tile([1, N], FP32)
                for c in range(2):
                    cs = slice(c * 512, (c + 1) * 512)
                    nc.scalar.activation(out=sq[:, cs], in_=xT[:, cs], func=AFT.Square)
                    nc.tensor.matmul(
                        px2[:, cs], lhsT=ones128, rhs=sq[:, cs], start=True, stop=True
                    )
                    nc.scalar.activation(
                        out=negx2[:, cs], in_=px2[:, cs], func=AFT.Copy, scale=-1.0
                    )

            e_sb = consts.tile([BL, GROUPS * GB * TOPK], FP32)
            nc.default_dma_engine.dma_start(out=e_sb, in_=e_in[:])
            fa_sb = consts.tile([BL, GROUPS * GB * TOPK], FP32)
            nc.default_dma_engine.dma_start(out=fa_sb, in_=fa_in[:])
            fd_sb = consts.tile([BL, GROUPS * GB * TOPK], FP32)
            nc.default_dma_engine.dma_start(out=fd_sb, in_=fd_in[:])
            g_sb = consts.tile([GB * TOPK, GB], FP32)
            nc.default_dma_engine.dma_start(out=g_sb, in_=g_in[:])

            # ---- Phase B: u = 2 Q X^T - ||x||^2, plus -u^T columns ----
            q2T = consts.tile([128, BL], FP32)   # (2Q)^T
            u_sb = consts.tile([BL, N], FP32)
            nuT = consts.tile([128, NBLK, BL], FP32)  # nuT[p, t, b] = -u[b, t*128+p]
            u_dram = dramp.tile([BL, N], FP32)
            with tc.tile_pool(name="ps_qt", bufs=2, space="PSUM") as ps_qt:
                pqt = ps_qt.tile([128, BL], FP32)
                nc.tensor.transpose(pqt, q_sb, ident[:BL, :BL])
                nc.scalar.activation(out=q2T, in_=pqt, func=AFT.Copy, scale=2.0)
                # nuT[:, t, b] = -u[b, t*128+p] computed directly:
                # uT_blk = xT_blk^T @ q2T + negx2_blk^T @ ones  (bit-identical
                # to the u_sb path: same products, same accumulation order).
                for t in range(NBLK):
                    put = ps_qt.tile([128, BL], FP32, tag="put")
                    nc.tensor.matmul(
                        put, lhsT=xT[:, t * 128 : (t + 1) * 128], rhs=q2T,
                        start=True, stop=False,
                    )
                    nc.tensor.matmul(
                        put, lhsT=negx2[:, t * 128 : (t + 1) * 128], rhs=ones1xb,
                        start=False, stop=True,
                    )
                    nc.vector.tensor_scalar(
                        out=nuT[:, t, :], in0=put, scalar1=-1.0, scalar2=None,
                        op0=ALU.mult,
                    )
                with tc.tile_pool(name="ps_u", bufs=1, space="PSUM") as ps_u:
                    pu = ps_u.tile([BL, N], FP32)
                    for c in range(2):
                        cs = slice(c * 512, (c + 1) * 512)
                        nc.tensor.matmul(
                            pu[:, cs], lhsT=q2T, rhs=xT[:, cs], start=True, stop=False
                        )
                        nc.tensor.matmul(
                            pu[:, cs], lhsT=ones1xb, rhs=negx2[:, cs],
                            start=False, stop=True,
                        )
                    nc.any.tensor_copy(u_sb, pu)
            nc.default_dma_engine.dma_start(out=u_dram[:4, :], in_=u_sb[:4, :])
            nc.scalar.dma_start(out=u_dram[4:, :], in_=u_sb[4:, :])
            Tb = small.tile([BL, 1], FP32, tag="Tb")
            nc.vector.tensor_reduce(
                out=Tb, in_=u_sb, axis=mybir.AxisListType.X, op=ALU.add
            )

            # Three-limb bf16 split of -nuT = u^T: u = hi + mid + lo exactly
            # (3 x 8 mantissa bits cover fp32's 24).  The 0/1 compare matrix G
            # is exact in bf16, so the TensorE reduce of (counts, L_hi, L_mid,
            # L_lo) runs at bf16 rate (1 col/cycle) instead of fp32's 4.
            hi_bf = consts.tile([128, NBLK, BL], BF16)
            nc.vector.tensor_scalar(out=hi_bf[:].rearrange("p t b -> p (t b)"),
                                    in0=nuT[:].rearrange("p t b -> p (t b)"),
                                    scalar1=-1.0, scalar2=None, op0=ALU.mult)
            hi32 = consts.tile([128, NBLK * BL], FP32)
            nc.vector.tensor_copy(hi32, hi_bf[:].rearrange("p t b -> p (t b)"))
            r1 = consts.tile([128, NBLK * BL], FP32)  # = hi - u
            nc.vector.tensor_tensor(out=r1, in0=nuT[:].rearrange("p t b -> p (t b)"),
                                    in1=hi32, op=ALU.add)
            mid_bf = consts.tile([128, NBLK, BL], BF16)
            nc.vector.tensor_scalar(out=mid_bf[:].rearrange("p t b -> p (t b)"),
                                    in0=r1, scalar1=-1.0, scalar2=None, op0=ALU.mult)
            mid32 = consts.tile([128, NBLK * BL], FP32)
            nc.vector.tensor_copy(mid32, mid_bf[:].rearrange("p t b -> p (t b)"))
            r2 = consts.tile([128, NBLK * BL], FP32)  # = hi + mid - u
            nc.vector.tensor_tensor(out=r2, in0=r1, in1=mid32, op=ALU.add)
            lo_bf = consts.tile([128, NBLK, BL], BF16)
            nc.vector.tensor_scalar(out=lo_bf[:].rearrange("p t b -> p (t b)"),
                                    in0=r2, scalar1=-1.0, scalar2=None, op0=ALU.mult)

            # W[p, t, b, m] (bf16): lhsT for the TensorE reduce of G.
            # col m==b: 1.0 -> counts r_j; m==32+b: hi -> L_hi row 32+b;
            # m==48+b: mid -> L_mid row 48+b; m==64+b: lo -> L_lo row 64+b.
            MW = 80
            W = consts.tile([128, NBLK, BL, MW], BF16)
            zb = consts.tile([128, 1], BF16)
            nc.vector.memset(zb, 0.0)
            zv = zb[:]
            zap = bass.AP(tensor=zv.tensor, offset=zv.offset,
                          ap=[zv.ap[0], [0, NBLK * BL * MW]])
            nc.gpsimd.affine_select(
                out=W[:].rearrange("p t b m -> p (t b m)"), in_=zap,
                compare_op=ALU.not_equal, fill=1.0, base=0,
                pattern=[[0, NBLK], [1, BL], [-1, MW]], channel_multiplier=0,
            )
            for t in range(NBLK):
                for lane, limb in ((32, hi_bf), (48, mid_bf), (64, lo_bf)):
                    lv = limb[:, t, :]
                    lb = bass.AP(tensor=lv.tensor, offset=lv.offset,
                                 ap=[lv.ap[0], list(lv.ap[1]), [0, BL]])
                    nc.gpsimd.tensor_tensor(
                        out=W[:, t, :, lane : lane + BL],
                        in0=W[:, t, :, 0:BL], in1=lb, op=ALU.mult,
                    )

            # ---- Phases C/D/E: pairwise passes, per-group overlap ----
            asumT = consts.tile([128, BL, NBLK], FP32)  # A_sum[b][t*128+p], ACT qs
            Ag0 = consts.tile([BL, N], FP32)
            Ag1 = consts.tile([BL, N], FP32)
            nc.gpsimd.memset(Ag0[:], 0.0)
            nc.gpsimd.memset(Ag1[:], 0.0)

            pm_cm = tc.tile_pool(name="ps_pm", bufs=1, space="PSUM")
            ps_pm = pm_cm.__enter__()
            po_cm = tc.tile_pool(name="ps_out", bufs=1, space="PSUM")
            ps_out = po_cm.__enter__()
            pm_tile = {}

            def phase_e_prelude(g):
                # pm = E . u can run as soon as u_sb is ready; F . A joins later
                pm = ps_pm.tile([GB * TOPK, N], FP32, tag="pm", name=f"pm{g}")
                pm_tile[g] = pm
                for c in range(2):
                    cs = slice(c * 512, (c + 1) * 512)
                    nc.tensor.matmul(
                        pm[:, cs], lhsT=e_sb[:, g * 80 : (g + 1) * 80],
                        rhs=u_sb[:, cs], start=True, stop=False,
                    )

            pa_cm, pa_tile, first = {}, {}, {}
            remaining = {0: len(G0_DVE) * NBLK, 1: len(G1_DVE) * NBLK}
            for g in (1, 0):  # stack allocator: group 0's pool closes first
                pa_cm[g] = tc.tile_pool(name=f"ps_pa{g}", bufs=1, space="PSUM")
                pool = pa_cm[g].__enter__()
                pa_tile[g] = pool.tile([MW, N], FP32, tag=f"pa{g}", name=f"pa{g}")
                first[g] = [True, True]

            def emit_act_query(b, ub):
                for t in range(NBLK):
                    sa = scrA.tile([128, N], FP32, tag="sa")
                    nc.scalar.activation(
                        out=sa, in_=ub, func=AFT.Abs,
                        bias=nuT[:, t, b : b + 1], scale=1.0,
                        accum_out=asumT[:, b, t : t + 1],
                    )

            def emit_dve_query(b, ub):
                g = 0 if b < GB else 1
                pa = pa_tile[g]
                for t in range(NBLK):
                    gps = b in GPS_SET or (b in TAIL_SPLIT and t >= 5)
                    eng = nc.gpsimd if gps else nc.vector
                    pool = scrP if gps else scrD
                    sd = pool.tile([128, N], BF16, tag="sdp" if gps else "sd", name="sd")
                    eng.tensor_scalar(
                        out=sd, in0=ub, scalar1=nuT[:, t, b : b + 1], scalar2=0.0,
                        op0=ALU.add, op1=ALU.is_gt,
                    )
                    remaining[g] -= 1
                    for c in range(2):
                        cs = slice(c * 512, (c + 1) * 512)
                        nc.tensor.matmul(
                            pa[:, cs], lhsT=W[:, t, b], rhs=sd[:, cs],
                            start=first[g][c], stop=remaining[g] == 0,
                        )
                        first[g][c] = False

            def combine_half(g, paw_g):
                # A[b,j] = u*(2r - N) + (T_b - 2(L_hi+L_mid+L_lo)), all 16 rows
                # (rows not hosted in this half read accumulated zeros -> junk,
                # only the hosted rows are consumed).  Group 0 (overlapped, not
                # latency-critical) sums the limb rows with accumulating SWDGE
                # DMAs; group 1 (the tail) uses three parallel DMAs on separate
                # issuers plus DVE adds.
                Lsum = consts.tile([BL, N], FP32, tag="Lsum")
                Lhi = consts.tile([BL, N], FP32, tag="Lhi")
                nc.sync.dma_start(out=Lhi, in_=paw_g[32 : 32 + BL, :])
                Lmid = consts.tile([BL, N], FP32, tag="Lmid")
                nc.scalar.dma_start(out=Lmid, in_=paw_g[48 : 48 + BL, :])
                Llo = consts.tile([BL, N], FP32, tag="Llo")
                nc.gpsimd.dma_start(out=Llo, in_=paw_g[64 : 64 + BL, :])
                tLa = cmbp.tile([BL, N], FP32, tag="cmbLa")
                nc.vector.tensor_tensor(out=tLa, in0=Lhi, in1=Lmid, op=ALU.add)
                nc.vector.tensor_tensor(out=Lsum, in0=tLa, in1=Llo, op=ALU.add)
                t1 = cmbp.tile([BL, N], FP32, tag="cmb1")
                nc.vector.tensor_scalar(
                    out=t1, in0=pa_tile[g][:BL, :], scalar1=2.0, scalar2=-float(N),
                    op0=ALU.mult, op1=ALU.add,
                )
                t2 = cmbp.tile([BL, N], FP32, tag="cmb2")
                nc.vector.tensor_tensor(out=t2, in0=t1, in1=u_sb, op=ALU.mult)
                t3 = cmbp.tile([BL, N], FP32, tag="cmb3")
                nc.vector.tensor_scalar(
                    out=t3, in0=Lsum, scalar1=-2.0, scalar2=Tb,
                    op0=ALU.mult, op1=ALU.add,
                )
                cmb = consts.tile([BL, N], FP32, tag=f"cmb{g}")
                nc.vector.tensor_tensor(out=cmb, in0=t2, in1=t3, op=ALU.add)
                return cmb

            def act_rows_to_ag(g, b0, nq, Ag):
                # DMA-transpose asumT[:, b0:b0+nq, :] into row form via a DRAM
                # bounce (src contiguity is 8-element runs along t -> cheap).
                adr = dramp.tile([nq, N], FP32, tag=f"adr{g}", name=f"adr{g}")
                dst = bass.AP(
                    tensor=adr[:].tensor, offset=adr[:].offset,
                    ap=[[1, 128], [N, nq], [128, NBLK]],
                )
                nc.sync.dma_start(out=dst, in_=asumT[:, b0 : b0 + nq, :])
                nc.scalar.dma_start(out=Ag[b0 : b0 + nq, :], in_=adr[:])

            def phase_e_act_part(g, Ag):
                # F.A contribution of the ACT-path rows; can run as soon as the
                # transposed ACT A-rows land, well before the DVE combine.
                pm = pm_tile[g]
                for c in range(2):
                    cs = slice(c * 512, (c + 1) * 512)
                    nc.tensor.matmul(
                        pm[:, cs], lhsT=fa_sb[:, g * 80 : (g + 1) * 80],
                        rhs=Ag[:, cs], start=False, stop=False,
                    )

            def phase_e(g, cmb):
                pm = pm_tile[g]
                for c in range(2):
                    cs = slice(c * 512, (c + 1) * 512)
                    nc.tensor.matmul(
                        pm[:, cs], lhsT=fd_sb[:, g * 80 : (g + 1) * 80],
                        rhs=cmb[:, cs], start=False, stop=True,
                    )
                nmx = small.tile([GB * TOPK, 1], FP32, tag="nmx")
                nc.vector.tensor_reduce(
                    out=nmx, in_=pm, axis=mybir.AxisListType.X,
                    op=ALU.max, negate=True,
                )
                exps = expp.tile([GB * TOPK, N], BF16, tag="exps")
                den = small.tile([GB * TOPK, 1], FP32, tag="den")
                nc.scalar.activation(
                    out=exps, in_=pm, func=AFT.Exp, bias=nmx, scale=1.0,
                    accum_out=den,
                )
                rden = small.tile([GB * TOPK, 1], FP32, tag="rden")
                nc.vector.reciprocal(rden, den)
                gr = small.tile([GB * TOPK, GB], BF16, tag="gr")
                nc.vector.tensor_scalar(
                    out=gr, in0=g_sb, scalar1=rden, scalar2=None, op0=ALU.mult
                )
                po = ps_out.tile([GB, N], FP32, tag="po", name=f"po{g}")
                og = expp.tile([GB, N], FP32, tag="og")
                for c in range(2):
                    cs = slice(c * 512, (c + 1) * 512)
                    nc.tensor.matmul(
                        po[:, cs], lhsT=gr, rhs=exps[:, cs], start=True, stop=True
                    )
                    nc.any.tensor_copy(og[:, cs], po[:, cs])
                    eng = nc.sync if c == 0 else nc.scalar
                    eng.dma_start(
                        out=out_t[g * GB : (g + 1) * GB, cs], in_=og[:, cs]
                    )

            def finalize_group(g):
                paw_g = consts.tile([MW, N], FP32, tag=f"paw{g}")
                nc.any.tensor_copy(paw_g, pa_tile[g])
                cmb = combine_half(g, paw_g)
                pa_cm[g].__exit__(None, None, None)
                phase_e(g, cmb)
                return paw_g

            paw0 = None
            for pi, (b0, b1) in enumerate(PAIRS):
                ub2 = bcast.tile([128, 2, N], FP32)
                base = u_dram[b0 : b0 + 1, :]
                src = bass.AP(
                    tensor=base.tensor, offset=base.offset,
                    ap=[[0, 128], [(b1 - b0) * N, 2], [1, N]],
                )
                nc.default_dma_engine.dma_start(out=ub2, in_=src)
                for k, b in enumerate((b0, b1)):
                    ub = ub2[:, k, :]
                    if b in ACT_SET:
                        emit_act_query(b, ub)
                    else:
                        emit_dve_query(b, ub)
                if pi == 2:
                    phase_e_prelude(0)
                    act_rows_to_ag(0, 0, 3, Ag0)
                    phase_e_act_part(0, Ag0)
                if pi == 3:
                    paw0 = finalize_group(0)
                if pi == 5:
                    phase_e_prelude(1)
                    act_rows_to_ag(1, 8, 2, Ag1)
                    phase_e_act_part(1, Ag1)
            finalize_group(1)
            po_cm.__exit__(None, None, None)
            pm_cm.__exit__(None, None, None)

            if debug_taps:
                nc.default_dma_engine.dma_start(out=dbg_u[:], in_=u_sb)
                nc.default_dma_engine.dma_start(out=dbg_a[:8], in_=Ag0[:8, :])
                nc.default_dma_engine.dma_start(out=dbg_a[8:], in_=Ag1[8:, :])
                nc.default_dma_engine.dma_start(
                    out=dbg_nut[:], in_=nuT[:].rearrange("p t b -> p (t b)")
                )
                nc.default_dma_engine.dma_start(out=dbg_paw[:], in_=paw0)

    nc.compile()
    return nc


_CACHE = {}


def _get_nc():
    if "nc" not in _CACHE:
        _CACHE["nc"] = _build_nc()
    return _CACHE["nc"]


def _in_maps(query, neighbors):
    query = np.ascontiguousarray(query, dtype=np.float32)
    neighbors = np.ascontiguousarray(neighbors, dtype=np.float32)
    return [
        {"query": query[c * BL : (c + 1) * BL], "neighbors": neighbors}
        for c in range(NCORES)
    ]


def _run(query, neighbors, **kw):
    nc = _get_nc()
    res = run_bass_kernel_spmd(nc, _in_maps(query, neighbors), list(range(NCORES)), **kw)
    out = np.concatenate([res.results[c]["out"] for c in range(NCORES)], axis=0)
    return out, res


def kernel(query, neighbors):
    out, _ = _run(query, neighbors)
    return out


def run_profiled(query, neighbors, **kw):
    out, res = _run(query, neighbors, trace=True, **kw)
    return out, res



# revision 50
# speedup vs baseline: 1.2373x; 1.2373x over previous
"""Trainium2 Bass kernel for deterministic NeuralSort soft-kNN (DKNN).

Math (per query b over N neighbors):
    s_j   = -||q_b - x_j||^2  ~  u_j = 2 q_b.x_j - ||x_j||^2  (||q||^2 cancels)
    A_j   = sum_i |u_j - u_i| = u_j(2 r_j - N) - 2 L_j + T
            r_j = #{i: u_i < u_j},  L_j = sum_{u_i < u_j} u_i,  T = sum_i u_i
    P[k,j]= softmax_j(scaling[k] * u_j - A_j),  k = 0..9
    out_j = sum_k P[k,j]

Sharding: data-parallel over B=128 queries across 8 cores (16 each).

Per-core structure:
  - ACT queries (exact-|.| path): ScalarE activation(Abs, bias=-u_p,
    accum_out) on a PSUM broadcast of u built by an exact one-hot matmul.
  - C queries (rank path): DVE/GpSimd tensor_scalar is_gt produces the 0/1
    comparison matrix C in fp8e4 (exact); TensorE DoubleRow fp8 matmuls
    reduce C against W8 = [count=1 | 4 fp8 limbs of u/4] giving counts r_j
    and limb partial sums; A is assembled inside the phase-E matmul.
  - Phase E per group of 8 queries: pm = E.u + CG.[A-rows | pa-rows],
    row-softmax via (max, exp-accum, reciprocal), out = G^T.exps.
"""

import numpy as np

import concourse.bass as bass
import concourse.bacc as bacc
import concourse.tile as tile
from concourse import mybir
from concourse.masks import make_identity
from concourse.bass_utils import run_bass_kernel_spmd

AFT = mybir.ActivationFunctionType
ALU = mybir.AluOpType
PM = mybir.MatmulPerfMode
FP32 = mybir.dt.float32
BF16 = mybir.dt.bfloat16
FP8 = mybir.dt.float8e4

B, N, D, TOPK = 128, 1024, 128, 10
USE_FP32R = False


def _R(ap):
    return ap.bitcast(mybir.dt.float32r) if USE_FP32R else ap

NCORES = 8
BL = B // NCORES          # 16 queries per core
NBLK = N // 128           # 8 row-blocks of the pairwise matrix
NPAIR = NBLK // 2         # 4 block-pairs for DoubleRow
GB = 8                    # queries per softmax group
NL = 2                    # bf16 limbs of u*LSCALE
LSCALE = 1.0              # limb scale (bf16 covers the range)

ACT_SET = (0, 1, 8, 9)                   # exact-Abs path queries
C_G0 = (2, 3, 4, 5, 6, 7)                # rank-path queries, group 0
C_G1 = (10, 11, 12, 13, 14, 15)          # rank-path queries, group 1
NC = 6                                   # C queries per group
PW = 1 + NL                              # pa rows per C query (count + limbs)
# processing order: g1 C first (its finalize must not be the tail), then g0 C.
C_ORDER = C_G1 + C_G0
ACT_ORDER = (0, 1, 8, 9)
# (query, pair) -> engine: gpsimd for these, DVE otherwise.  First queries
# lean DVE (gpsimd starts late), middle split 2/2, last queries split so the
# tail drains on both engines.
POOL_PAIRS = ({(10, 3), (11, 3), (12, 3), (13, 3)}
              | {(b, k) for b in (14, 15, 2, 3, 4, 5, 6, 7) for k in (1, 3)})


def _host_consts():
    scaling = (N + 1 - 2.0 * (np.arange(TOPK) + 1)).astype(np.float32)
    # E[b, g*80 + (b%8)*10 + k] = scaling[k] for b in group g
    E = np.zeros((BL, 2 * GB * TOPK), np.float32)
    for b in range(BL):
        g = b // GB
        sc = scaling + (0.0 if b in ACT_SET else float(N))
        E[b, g * 80 + (b % GB) * TOPK: g * 80 + (b % GB) * TOPK + TOPK] = sc
    # pm rows (b,k) = (scaling[k] + N)*u - 2*(r.u) - T + (2/LSCALE)*sum_l Ls
    # for C queries; scaling[k]*u - A for ACT queries.
    # comb layout [70, N]: rows 0..5 = -2*(r.u) (coef +1); rows 6..7 ACT A
    # (coef -1); rows 32..37 counts (0); 38..61 limb sums (+2/LSCALE);
    # rows 64..69 T-rows (coef -1).
    CG = np.zeros((70, GB * TOPK), np.float32)
    for ci in range(NC):
        CG[ci, (ci + 2) * TOPK: (ci + 3) * TOPK] = 1.0
        CG[64 + ci, (ci + 2) * TOPK: (ci + 3) * TOPK] = -1.0
    for a8 in range(2):
        CG[6 + a8, a8 * TOPK: (a8 + 1) * TOPK] = -1.0
    for ci in range(NC):
        for l in range(NL):
            CG[32 + NC + ci * NL + l, (ci + 2) * TOPK: (ci + 3) * TOPK] = (
                2.0 / LSCALE)
    # G[bl*10+k, bl] = 1 (softmax-row -> query-row collapse)
    G = np.zeros((GB * TOPK, GB), np.float32)
    for bl in range(GB):
        G[bl * TOPK: (bl + 1) * TOPK, bl] = 1.0
    # one-hot row selectors for the ACT ub broadcast: SEL[q, ai*128+p] = 1[q==ACT_ORDER[ai]]
    SEL = np.zeros((BL, len(ACT_ORDER) * 128), np.float32)
    for ai, b in enumerate(ACT_ORDER):
        SEL[b, ai * 128: (ai + 1) * 128] = 1.0
    # pack all consts into one [80, 760] payload (one DMA)
    PK = np.zeros((80, 760), np.float32)
    PK[:BL, 0:160] = E
    PK[:70, 160:240] = CG
    PK[:80, 240:248] = G
    PK[:BL, 248:760] = SEL
    return PK


def _build_nc(debug_taps=False):
    nc = bacc.Bacc(None, target_bir_lowering=False)

    q_in = nc.dram_tensor("query", [BL, D], FP32, kind="ExternalInput")
    x_in = nc.dram_tensor("neighbors", [N, D], FP32, kind="ExternalInput")
    out_t = nc.dram_tensor("out", [BL, N], FP32, kind="ExternalOutput")
    if debug_taps:
        dbg_u = nc.dram_tensor("dbg_u", [BL, N], FP32, kind="ExternalOutput")
        dbg_cmb0 = nc.dram_tensor("dbg_cmb0", [70, N], FP32,
                                  kind="ExternalOutput")
        dbg_cmb1 = nc.dram_tensor("dbg_cmb1", [70, N], FP32,
                                  kind="ExternalOutput")
        dbg_w8 = nc.dram_tensor("dbg_w8", [128, NPAIR * 2 * NC * 2 * NC * PW],
                                mybir.dt.float8e4, kind="ExternalOutput")
        dbg_sd = nc.dram_tensor("dbg_sd", [128, 2 * N], mybir.dt.float8e4,
                                kind="ExternalOutput")

    PK = _host_consts()
    pk_in = nc.inline_tensor(PK, "lhs_pk")

    with tile.TileContext(nc) as tc:
        with (
            tc.tile_pool(name="consts", bufs=1) as consts,
            tc.tile_pool(name="xp", bufs=1) as xp,
            tc.tile_pool(name="bcast", bufs=6) as bcast,
            tc.tile_pool(name="scrA", bufs=4) as scrA,
            tc.tile_pool(name="scrD", bufs=8) as scrD,
            tc.tile_pool(name="scrP", bufs=5) as scrP,
            tc.tile_pool(name="small", bufs=8) as small,
            tc.tile_pool(name="dramp", bufs=1, space="DRAM") as dramp,
        ):
            ident = consts.tile([128, 128], FP32)
            make_identity(nc, ident)
            mones = consts.tile([128, BL], FP32)
            nc.vector.memset(mones, -1.0)

            # ---- Phase A: neighbors in, transpose to [d, j] ----
            q_sb = small.tile([BL, D], FP32)
            nc.default_dma_engine.dma_start(out=q_sb, in_=q_in[:])
            x_sb = xp.tile([128, NBLK, D], FP32)
            xv = x_in[:].rearrange("(t p) d -> p t d", p=128)
            for qtr in range(4):
                nc.default_dma_engine.dma_start(
                    out=x_sb[:, qtr * 2: qtr * 2 + 2, :],
                    in_=xv[:, qtr * 2: qtr * 2 + 2, :])
            # packed consts (needed from first ubp matmul onward)
            pk_sb = consts.tile([80, 760], FP32)
            nc.sync.dma_start(out=pk_sb, in_=pk_in[:])
            e_sb = pk_sb[:BL, 0:160]
            cg_sb = pk_sb[:70, 160:240]
            g_sb = pk_sb[:80, 240:248]
            sel_sb = pk_sb[:BL, 248:760]

            q2T = consts.tile([128, BL], FP32)   # (2Q)^T
            xT = xp.tile([128, N], FP32)  # xT[d, j] = X[j, d]
            with tc.tile_pool(name="ps_tr", bufs=4, space="PSUM") as ps_tr:
                pqt = ps_tr.tile([128, BL], FP32, tag="pqt", name="pqt")
                nc.tensor.transpose(pqt, q_sb, ident[:BL, :BL])
                nc.scalar.activation(out=q2T, in_=pqt, func=AFT.Copy, scale=2.0)
                for t in range(NBLK):
                    ptr = ps_tr.tile([128, 128], FP32)
                    nc.tensor.transpose(ptr, x_sb[:, t, :], ident)
                    nc.scalar.activation(out=xT[:, t * 128: (t + 1) * 128],
                                         in_=ptr, func=AFT.Copy)

            sq = xp.tile([128, N], FP32)
            for c in range(2):
                cs = slice(c * 512, (c + 1) * 512)
                nc.scalar.activation(out=sq[:, cs], in_=xT[:, cs], func=AFT.Square)

            # ---- Phase B: u = 2 Q X^T - ||x||^2; nuT = -u^T; fp8 limbs ----
            u_sb = consts.tile([BL, N], FP32)
            nuT = consts.tile([128, NBLK, BL], FP32)  # nuT[p, t, b] = -u[b, t*128+p]
            with tc.tile_pool(name="ps_qt", bufs=2, space="PSUM") as ps_qt:
                u_dram_a = dramp.tile([BL, 512], FP32, tag="uda", name="uda")
                u_dram_b = dramp.tile([BL, 512], FP32, tag="udb", name="udb")
                u_dram = dramp.tile([BL, N], FP32)
                with tc.tile_pool(name="ps_u", bufs=1, space="PSUM") as ps_u:
                    pu = ps_u.tile([BL, N], FP32)
                    for c in range(2):
                        cs = slice(c * 512, (c + 1) * 512)
                        nc.tensor.matmul(
                            pu[:, cs], lhsT=_R(q2T[:]), rhs=_R(xT[:, cs]),
                            start=True, stop=False,
                        )
                        nc.tensor.matmul(
                            pu[:, cs], lhsT=_R(mones[:]), rhs=_R(sq[:, cs]),
                            start=False, stop=True,
                        )
                        nc.scalar.activation(out=u_sb[:, cs], in_=pu[:, cs],
                                             func=AFT.Copy)
                    # nuT via exact transposes of u_sb (bit-identical to u)
                    for t in range(NBLK):
                        put = ps_qt.tile([128, BL], FP32, tag="put")
                        nc.tensor.transpose(
                            put, u_sb[:, t * 128: (t + 1) * 128], ident[:BL, :BL]
                        )
                        nc.scalar.activation(
                            out=nuT[:, t, :], in_=put, func=AFT.Copy, scale=-1.0,
                        )



            # W8[p, t, ci, col] fp8 lhsT blocks: for C query ci of group g,
            # slice [:, t, ci, g*CW:(g+1)*CW] has count 1.0 at col g*CW+ci and
            # limb l of u*LSCALE at col g*CW+NC+ci*NL+l; other cols zero
            # (zeros separate queries in the shared pa accumulator).
            CW = NC * PW  # 30 cols per group
            W8 = consts.tile([128, NBLK, NC, 2 * CW], BF16)
            nc.vector.memset(W8[:].rearrange("p t c m -> p (t c m)"), 0.0)
            wv = W8[:]
            for g in range(2):
                cap = bass.AP(tensor=wv.tensor, offset=wv.offset + g * CW,
                              ap=[wv.ap[0], [2 * NC * CW, NBLK],
                                  [2 * CW + 1, NC]])
                nc.vector.memset(cap, 1.0)
            # limb chain: r = u*LSCALE - sum(limbs so far); limbs8_l exact fp8
            rres = consts.tile([128, NBLK, BL], FP32)
            nc.vector.tensor_scalar(
                out=rres[:].rearrange("p t b -> p (t b)"),
                in0=nuT[:].rearrange("p t b -> p (t b)"),
                scalar1=-LSCALE, scalar2=None, op0=ALU.mult,
            )
            limbs8 = consts.tile([128, NL, NBLK, BL], BF16)
            for l in range(NL):
                nc.vector.tensor_copy(
                    limbs8[:, l].rearrange("p t b -> p (t b)"),
                    rres[:].rearrange("p t b -> p (t b)"),
                )
                if l < NL - 1:
                    nc.vector.tensor_tensor(
                        out=rres[:].rearrange("p t b -> p (t b)"),
                        in0=rres[:].rearrange("p t b -> p (t b)"),
                        in1=limbs8[:, l].rearrange("p t b -> p (t b)"),
                        op=ALU.subtract,
                    )
                for g, cset in enumerate((C_G0, C_G1)):
                    b0 = cset[0]
                    wap = bass.AP(
                        tensor=wv.tensor,
                        offset=wv.offset + g * CW + NC + l,
                        ap=[wv.ap[0], [2 * NC * CW, NBLK], [2 * CW + NL, NC]],
                    )
                    lv = limbs8[:, l]
                    lap = bass.AP(
                        tensor=lv.tensor, offset=lv.offset + b0,
                        ap=[lv.ap[0], [BL, NBLK], [1, NC]],
                    )
                    nc.vector.tensor_copy(wap, lap)

            # ---- mid phase: comparisons + DoubleRow reduce + Abs path ----
            comb, uC, TbC = {}, {}, {}
            for g in range(2):
                comb[g] = consts.tile([70, N], FP32, tag=f"comb{g}",
                                      name=f"comb{g}")
                nc.vector.memset(comb[g][:], 0.0)
                uC[g] = consts.tile([NC, N], FP32, tag=f"uC{g}", name=f"uC{g}")
                TbC[g] = small.tile([NC, 1], FP32, tag=f"TbC{g}", name=f"TbC{g}")

            def emit_tb():
                # emitted after the limb chain: Tb on DVE, then staging DMAs
                Tb = small.tile([BL, 1], FP32, tag="Tb")
                nc.vector.tensor_reduce(
                    out=Tb, in_=u_sb, axis=mybir.AxisListType.X, op=ALU.add
                )
                Tb_dram = dramp.tile([BL, 1], FP32, tag="Tb_dram")
                nc.sync.dma_start(out=Tb_dram[:], in_=Tb)
                zeros6 = consts.tile([NC, N], FP32, tag="zeros6")
                nc.vector.memset(zeros6, 0.0)
                for g in range(2):
                    b0 = (C_G0 if g == 0 else C_G1)[0]
                    nc.sync.dma_start(out=uC[g], in_=u_dram[b0: b0 + NC, :])
                    nc.sync.dma_start(out=TbC[g], in_=Tb_dram[b0: b0 + NC, :])
                    for c in range(2):
                        cs = slice(c * 512, (c + 1) * 512)
                        eng = nc.vector if c == 0 else nc.gpsimd
                        eng.tensor_scalar(
                            out=comb[g][64: 64 + NC, cs], in0=zeros6[:, cs],
                            scalar1=TbC[g], scalar2=None, op0=ALU.add,
                        )

            pa_cm, pa_tile = {}, {}
            for g in (0, 1):  # stack: g1 on top, closed first (finalizes first)
                pa_cm[g] = tc.tile_pool(name=f"ps_pa{g}", bufs=1, space="PSUM")
                pool = pa_cm[g].__enter__()
                pa_tile[g] = pool.tile([PW * NC, N], FP32, tag=f"pa{g}", name=f"pa{g}")

            remaining = {0: NC * NPAIR, 1: NC * NPAIR}
            first = {0: [True, True], 1: [True, True]}
            ub_map = {}

            def load_ub_pair(b1, b2, split=False, skip_udram=False):
                ub2 = bcast.tile([128, 2, N], FP32, tag="ub2")
                if split:
                    # per-chunk DRAM tiles break the false whole-tile dep:
                    # each broadcast half launches as soon as its chunk lands
                    for c, ud in ((0, u_dram_a), (1, u_dram_b)):
                        cs = slice(c * 512, (c + 1) * 512)
                        if not skip_udram:
                            nc.sync.dma_start(out=ud[:], in_=u_sb[:, cs])
                        src = ud[b1: b1 + 1, :]
                        bsrc = bass.AP(
                            tensor=src.tensor, offset=src.offset,
                            ap=[[0, 128], [(b2 - b1) * 512, 2], [1, 512]])
                        nc.sync.dma_start(out=ub2[:, :, cs], in_=bsrc)
                    if not skip_udram:
                        nc.sync.dma_start(out=u_dram[:], in_=u_sb[:])
                else:
                    src = u_dram[b1: b1 + 1, :]
                    bsrc = bass.AP(tensor=src.tensor, offset=src.offset,
                                   ap=[[0, 128], [(b2 - b1) * N, 2], [1, N]])
                    nc.sync.dma_start(out=ub2, in_=bsrc)
                ub_map[b1] = ub2[:, 0, :]
                ub_map[b2] = ub2[:, 1, :]

            def emit_c_query(b):
                g = 0 if b < GB else 1
                ci = b - (C_G0[0] if g == 0 else C_G1[0])
                ub = ub_map[b]
                pa = pa_tile[g]
                for k in range(NPAIR):
                    gps = (b, k) in POOL_PAIRS
                    eng = nc.gpsimd if gps else nc.vector
                    pool = scrP if gps else scrD
                    sd8 = pool.tile([128, 2, N], BF16,
                                    tag="sd8p" if gps else "sd8", name="sd8")
                    for sub in range(2):
                        t = 2 * k + sub
                        if b in (10, 11):
                            # first-arriving pair: chunked so compares start
                            # on the first broadcast half
                            for c in range(2):
                                cs = slice(c * 512, (c + 1) * 512)
                                eng.tensor_scalar(
                                    out=sd8[:, sub, cs], in0=ub[:, cs],
                                    scalar1=nuT[:, t, b: b + 1], scalar2=0.0,
                                    op0=ALU.add, op1=ALU.is_gt,
                                )
                        else:
                            eng.tensor_scalar(
                                out=sd8[:, sub, :], in0=ub,
                                scalar1=nuT[:, t, b: b + 1], scalar2=0.0,
                                op0=ALU.add, op1=ALU.is_gt,
                            )
                    if debug_taps and b == 2 and k == 0:
                        nc.sync.dma_start(
                            out=dbg_sd[:],
                            in_=sd8[:].rearrange("p a b -> p (a b)"))
                    remaining[g] -= 1
                    g0 = 0 if g == 0 else CW
                    for sub in range(2):
                        for c in range(2):
                            cs = slice(c * 512, (c + 1) * 512)
                            nc.tensor.matmul(
                                pa[:, cs],
                                lhsT=W8[:, 2 * k + sub, ci, g0: g0 + CW],
                                rhs=sd8[:, sub, cs], start=first[g][c],
                                stop=remaining[g] == 0 and sub == 1,
                            )
                            first[g][c] = False

            def emit_act_query(ai, b):
                g = 0 if b < GB else 1
                ubp = ub_map[b]
                asum = small.tile([128, NBLK], FP32, tag=f"asum{ai}", name=f"asum{ai}")
                for t in range(NBLK):
                    sa = scrA.tile([128, N], FP32, tag="sa")
                    nc.scalar.activation(
                        out=sa, in_=ubp, func=AFT.Abs,
                        bias=nuT[:, t, b: b + 1], scale=1.0,
                        accum_out=asum[:, t: t + 1],
                    )
                # A row -> comb[g] row b%8 (transposed store, DRAM bounce)
                adr = dramp.tile([1, N], FP32, tag=f"adr{ai}", name=f"adr{ai}")
                dv = adr[:]
                dap = bass.AP(tensor=dv.tensor, offset=dv.offset,
                              ap=[[1, 128], [128, NBLK]])
                nc.sync.dma_start(out=dap, in_=asum[:])
                nc.sync.dma_start(
                    out=comb[g][6 + (b % GB): 7 + (b % GB), :], in_=adr[:])

            fin_state = {}

            def pm_open(g):
                side = "right" if g == 0 else "left"
                pm_cm = tc.tile_pool(name=f"ps_pm{g}", bufs=1, space="PSUM",
                                     side=side)
                ps_pm = pm_cm.__enter__()
                pm = ps_pm.tile([GB * TOPK, N], FP32, tag=f"pm{g}", name=f"pm{g}")
                for c in range(2):
                    cs = slice(c * 512, (c + 1) * 512)
                    nc.tensor.matmul(
                        pm[:, cs], lhsT=_R(e_sb[:, g * 80: (g + 1) * 80]),
                        rhs=_R(u_sb[:, cs]), start=True, stop=False,
                    )
                fin_state[g] = (pm_cm, pm)

            def comb_build(g):
                pa = pa_tile[g]
                # Pool: pa rows -> comb rows 32..61 (both chunks), then the
                # c1 A'-chain; DVE: the c0 A'-chain in parallel.
                nc.scalar.activation(out=comb[g][32: 32 + PW * NC, :512],
                                     in_=pa[:, :512], func=AFT.Copy)
                nc.scalar.activation(out=comb[g][32: 32 + PW * NC, 512:],
                                     in_=pa[:, 512:], func=AFT.Copy)
                for c in range(2):
                    cs = slice(c * 512, (c + 1) * 512)
                    nc.vector.scalar_tensor_tensor(
                        out=comb[g][0: NC, cs], in0=pa[:NC, cs], scalar=-2.0,
                        in1=uC[g][:, cs], op0=ALU.mult, op1=ALU.mult,
                    )
                pa_cm[g].__exit__(None, None, None)

            def finalize_early(g):
                comb_build(g)
                pm_open(g)

            def finalize_late(g):
                pm_cm, pm = fin_state[g]
                mx = small.tile([GB * TOPK, 2], FP32, tag=f"mx{g}")
                for c in range(2):
                    cs = slice(c * 512, (c + 1) * 512)
                    nc.tensor.matmul(
                        pm[:, cs], lhsT=_R(cg_sb[:]),
                        rhs=_R(comb[g][:, cs]), start=False, stop=True,
                    )
                    nc.vector.tensor_reduce(
                        out=mx[:, c: c + 1], in_=pm[:, cs],
                        axis=mybir.AxisListType.X, op=ALU.max,
                    )
                nmx = small.tile([GB * TOPK, 1], FP32, tag=f"nmx{g}")
                nc.vector.tensor_reduce(
                    out=nmx, in_=mx, axis=mybir.AxisListType.X, op=ALU.max,
                    negate=True,
                )
                exps = consts.tile([GB * TOPK, N], BF16, tag=f"exps{g}")
                den = small.tile([GB * TOPK, 1], FP32, tag=f"den{g}")
                nc.scalar.activation(
                    out=exps, in_=pm, func=AFT.Exp, bias=nmx, scale=1.0,
                    accum_out=den,
                )
                pm_cm.__exit__(None, None, None)
                rden = small.tile([GB * TOPK, 1], FP32, tag=f"rden{g}")
                nc.vector.reciprocal(rden, den)
                gr = small.tile([GB * TOPK, GB], BF16, tag=f"gr{g}")
                nc.vector.tensor_scalar(
                    out=gr, in0=g_sb, scalar1=rden, scalar2=None, op0=ALU.mult
                )
                po_cm = tc.tile_pool(name=f"ps_po{g}", bufs=1, space="PSUM")
                ps_po = po_cm.__enter__()
                po = ps_po.tile([GB, N], FP32, tag=f"po{g}", name=f"po{g}")
                og = consts.tile([GB, N], FP32, tag=f"og{g}")
                for c in range(2):
                    cs = slice(c * 512, (c + 1) * 512)
                    nc.tensor.matmul(
                        po[:, cs], lhsT=gr, rhs=exps[:, cs], start=True, stop=True
                    )
                    if c == 0:
                        nc.scalar.activation(out=og[:, cs], in_=po[:, cs],
                                             func=AFT.Copy)
                    else:
                        nc.vector.tensor_copy(og[:, cs], po[:, cs])
                    nc.sync.dma_start(
                        out=out_t[g * GB: (g + 1) * GB, cs], in_=og[:, cs]
                    )
                po_cm.__exit__(None, None, None)

            # schedule: all broadcasts issue upfront (SP in-order, no gating
            # deps beyond u_dram); g1 C queries first so group 1 finalizes
            # mid-kernel; ACT g0 queries last -> group 0 tail is A-row gated.
            load_ub_pair(10, 11, split=True)
            load_ub_pair(0, 1, split=True, skip_udram=True)
            load_ub_pair(12, 13)
            load_ub_pair(8, 9)
            load_ub_pair(14, 15)
            load_ub_pair(2, 3)
            load_ub_pair(4, 5)
            load_ub_pair(6, 7)
            emit_tb()
            emit_c_query(10)
            emit_c_query(11)
            emit_act_query(0, 0)
            emit_c_query(12)
            emit_c_query(13)
            emit_act_query(1, 1)
            emit_c_query(14)
            emit_c_query(15)
            emit_act_query(2, 8)
            emit_c_query(2)
            finalize_early(1)
            emit_c_query(3)
            emit_act_query(3, 9)
            pm_open(0)
            emit_c_query(4)
            emit_c_query(5)
            emit_c_query(6)
            emit_c_query(7)
            finalize_late(1)
            comb_build(0)
            finalize_late(0)

            if debug_taps:
                nc.default_dma_engine.dma_start(
                    out=dbg_w8[:], in_=W8[:].rearrange("p a s c m -> p (a s c m)"))
                nc.default_dma_engine.dma_start(out=dbg_u[:], in_=u_sb)
                nc.default_dma_engine.dma_start(out=dbg_cmb0[:], in_=comb[0])
                nc.default_dma_engine.dma_start(out=dbg_cmb1[:], in_=comb[1])

    nc.compile()
    return nc


_CACHE = {}


def _get_nc(**kw):
    key = tuple(sorted(kw.items()))
    if key not in _CACHE:
        _CACHE[key] = _build_nc(**kw)
    return _CACHE[key]


def _in_maps(query, neighbors):
    query = np.ascontiguousarray(query, dtype=np.float32)
    neighbors = np.ascontiguousarray(neighbors, dtype=np.float32)
    return [
        {"query": query[c * BL: (c + 1) * BL], "neighbors": neighbors}
        for c in range(NCORES)
    ]


def _run(query, neighbors, **kw):
    nc = _get_nc()
    res = run_bass_kernel_spmd(nc, _in_maps(query, neighbors), list(range(NCORES)), **kw)
    out = np.concatenate([res.results[c]["out"] for c in range(NCORES)], axis=0)
    return out, res


def kernel(query, neighbors):
    out, _ = _run(query, neighbors)
    return out


def run_profiled(query, neighbors, **kw):
    out, res = _run(query, neighbors, trace=True, **kw)
    return out, res
